# revision 28
# baseline (speedup 1.0000x reference)
"""NeuroSAT message-passing kernel for 8 Trainium2 NeuronCores (Bass/Tile).

Strategy
--------
The dense adjacency factors as A = D_row @ B @ D_col with B binary, so B
streams from HBM in fp8 (1.0/0.0 are exact in e4m3) as the *moving* matmul
operand against bf16 stationary message tiles; the degree scalings are
per-partition activation scales / free tensor_tensor multiplies at PSUM
eviction, and the (scaling-entangled) final-layer MLP biases become rank-1
K=1 matmul corrections accumulated straight into the LSTM gate PSUMs.

Sharding (8 cores):
  - clauses: core k owns [2048k, 2048k+2048)
  - literals: core k owns [512k, 512k+512) u [4096+512k, 4096+512k+512)
    (a positive block and its negation block, so NeuroSAT's "flip" is a
    local slice swap instead of a cross-core exchange)
All row-wise ops (MLPs, LSTMs) run on the local shard in feature-major
layout [dim(128) x rows]; the two A-applications per round contract over
the full lit/clause axes, fed by AllGathers of the scaled row-major
L/C messages. Each AllGather is split in two and the contraction loops
are ordered chunk-major so collectives and B-streaming DMAs hide under
the previous chunk's matmuls (keeps TensorE warm through the round).
"""
import sys

sys.path.insert(0, "/opt/trn_rl_repo")

import numpy as np
import ml_dtypes

import concourse.bass as bass
import concourse.mybir as mybir
import concourse.tile as tile
from concourse import bacc
from concourse import bass_utils

dt = mybir.dt
AF = mybir.ActivationFunctionType
ALU = mybir.AluOpType
bf16 = ml_dtypes.bfloat16
f8 = ml_dtypes.float8_e4m3

NCORES = 8
D = 128
NL_TOT, NCL_TOT, NV = 8192, 16384, 4096
NL = NL_TOT // NCORES      # 1024 lits per core
NCL = NCL_TOT // NCORES    # 2048 clauses per core
KT1 = NL_TOT // 128        # 64  k-tiles for A^T @ Lpre
KT2 = NCL_TOT // 128       # 128 k-tiles for A @ Cpre
FP8_ONE = 0x38             # bit pattern of 1.0 in float8_e4m3
GAIN = np.float32(128.0)   # power-of-2 pre-scale keeping fp8 messages normal-range


# ---------------------------------------------------------------------------
# device program
# ---------------------------------------------------------------------------

def build_program(rounds: int):
    nc = bacc.Bacc("TRN2", target_bir_lowering=False, debug=False,
                   num_devices=NCORES)

    def inp(name, shape, dty):
        return nc.dram_tensor(name, list(shape), dty, kind="ExternalInput")

    # B streams, pre-tiled to match the chunk-major contraction loops:
    # b1[nn, h]: slab of 32 k-tiles [128p, 32tt, 512c] covering clause chunk
    #            nn, lit-tile phase h (tt = 4k + jj, global tile t = 8k+4h+jj)
    # b2[nn, h, s]: slab of 32 k-tiles [128p, 32tt, 512l] covering lit chunk
    #            nn, clause-tile phase h, half s (tt=8kk+j2, T=16(4s+kk)+8h+j2)
    b1 = inp("b1", [4, 2, 128, 32 * 512], dt.float8e4)
    b2 = inp("b2", [2, 2, 2, 128, 32 * 512], dt.float8e4)
    w = {}
    for p in ("lm", "cm", "lv"):
        for l in ("w1t", "w2t", "w3t"):
            shape = [128, 1] if (p, l) == ("lv", "w3t") else [128, 128]
            w[f"{p}_{l}"] = inp(f"{p}_{l}", shape, dt.bfloat16)
        for l in ("b1", "b2"):
            w[f"{p}_{l}"] = inp(f"{p}_{l}", [128, 1], dt.float32)
    cu_wt = inp("cu_wt", [128, 512], dt.bfloat16)      # cu_wih.T
    cu_ut = inp("cu_ut", [128, 512], dt.bfloat16)      # cu_whh.T
    cu_b = inp("cu_b", [128, 4], dt.float32)
    lu_wcl = inp("lu_wcl", [128, 512], dt.bfloat16)    # lu_wih[:, :128].T
    lu_wfl = inp("lu_wfl", [128, 512], dt.bfloat16)    # lu_wih[:, 128:].T
    lu_ut = inp("lu_ut", [128, 512], dt.bfloat16)      # lu_whh.T
    lu_b = inp("lu_b", [128, 4], dt.float32)
    u1 = inp("u1", [1, 512], dt.bfloat16)              # cu_wih @ lm_b3
    u2 = inp("u2", [1, 512], dt.bfloat16)              # lu_wih[:, :128] @ cm_b3
    scol = inp("scol", [1, NCL], dt.bfloat16)          # col * (row @ B), my clauses
    srow = inp("srow", [1, NL], dt.bfloat16)           # row * (B @ col), my lits
    colb = inp("colb", [128, NCL], dt.bfloat16)         # col bcast over partitions
    rowb = inp("rowb", [128, NL], dt.bfloat16)          # row bcast over partitions
    rowsc = inp("rowsc", [128, 8], dt.float32)         # row, per lit-tile column
    colsc = inp("colsc", [128, 16], dt.float32)        # col, per clause-tile column
    lh0 = inp("lh0", [128, NL], dt.bfloat16)
    ch0 = inp("ch0", [128, NCL], dt.bfloat16)

    vote_out = nc.dram_tensor("vote", [1, NL], dt.float32, kind="ExternalOutput")
    ag1_out = nc.dram_tensor("ag1_out", [NCORES, 128, NL], dt.float8e4,
                             addr_space="Shared")
    ag2_out = nc.dram_tensor("ag2_out", [NCORES, 128, NCL], dt.float8e4,
                             addr_space="Shared")
    rg = [list(range(NCORES))]

    with tile.TileContext(nc) as tc:
        with (
            tc.tile_pool(name="const", bufs=1) as cp,
            tc.tile_pool(name="state", bufs=1) as sp,
            tc.tile_pool(name="work", bufs=1) as wp,
            tc.tile_pool(name="chunk", bufs=2) as kp,
            tc.tile_pool(name="bstream", bufs=4) as bp,
            tc.tile_pool(name="psd", bufs=1, space="PSUM") as psd,
            tc.tile_pool(name="psg", bufs=1, space="PSUM") as psg,
            tc.tile_pool(name="psm", bufs=2, space="PSUM") as psm,
            tc.tile_pool(name="dram", bufs=1, space="DRAM") as dp,
        ):
            # ---- constants into SBUF ----
            C = {}
            for name, t in [
                ("cu_wt", cu_wt), ("cu_ut", cu_ut), ("lu_wcl", lu_wcl),
                ("lu_wfl", lu_wfl), ("lu_ut", lu_ut),
            ]:
                C[name] = cp.tile([128, 512], dt.bfloat16, name=name)
                nc.scalar.dma_start(out=C[name], in_=t.ap())
            for p in ("lm", "cm", "lv"):
                for l in ("w1t", "w2t", "w3t"):
                    shape = [128, 1] if (p, l) == ("lv", "w3t") else [128, 128]
                    C[f"{p}_{l}"] = cp.tile(shape, dt.bfloat16, name=f"{p}_{l}")
                    nc.scalar.dma_start(out=C[f"{p}_{l}"], in_=w[f"{p}_{l}"].ap())
                for l in ("b1", "b2"):
                    C[f"{p}_{l}"] = cp.tile([128, 1], dt.float32, name=f"{p}_{l}")
                    nc.scalar.dma_start(out=C[f"{p}_{l}"], in_=w[f"{p}_{l}"].ap())
            for name, t, shape, dty in [
                ("cu_b", cu_b, [128, 4], dt.float32),
                ("lu_b", lu_b, [128, 4], dt.float32),
                ("u1", u1, [1, 512], dt.bfloat16),
                ("u2", u2, [1, 512], dt.bfloat16),
                ("scol", scol, [1, NCL], dt.bfloat16),
                ("srow", srow, [1, NL], dt.bfloat16),
                ("colb", colb, [128, NCL], dt.bfloat16),
                ("rowb", rowb, [128, NL], dt.bfloat16),
                ("rowsc", rowsc, [128, 8], dt.float32),
                ("colsc", colsc, [128, 16], dt.float32),
            ]:
                C[name] = cp.tile(shape, dty, name=name)
                nc.scalar.dma_start(out=C[name], in_=t.ap())

            # ---- states ----
            Lh_pp = [sp.tile([128, NL], dt.bfloat16, name="Lh_a"),
                     sp.tile([128, NL], dt.bfloat16, name="Lh_b")]
            Ch = sp.tile([128, NCL], dt.bfloat16, name="Ch")
            Lc = sp.tile([128, NL], dt.float32, name="Lc")
            Cc = sp.tile([128, NCL], dt.float32, name="Cc")
            nc.scalar.dma_start(out=Lh_pp[0], in_=lh0.ap())
            nc.scalar.dma_start(out=Ch, in_=ch0.ap())
            nc.vector.memset(Lc, 0.0)
            nc.vector.memset(Cc, 0.0)

            # ---- round-persistent work tiles ----
            lpre_img = wp.tile([128, NL], dt.float8e4, name="lpre_img")
            cpre_img = wp.tile([128, NCL], dt.float8e4, name="cpre_img")
            lpre_full = wp.tile([128, NL_TOT], dt.float8e4, name="lpre_full")
            cpre_full = wp.tile([128, NCL_TOT], dt.float8e4, name="cpre_full")
            lcs = wp.tile([128, NCL], dt.bfloat16, name="lcs")
            cls = wp.tile([128, NL], dt.bfloat16, name="cls")
            h1l = wp.tile([128, NL], dt.bfloat16, name="h1l")
            h2l = wp.tile([128, NL], dt.bfloat16, name="h2l")
            h1c = wp.tile([128, NCL], dt.bfloat16, name="h1c")
            h2c = wp.tile([128, NCL], dt.bfloat16, name="h2c")
            ag1_in = dp.tile([128, NL], dt.float8e4, name="ag1_in")
            ag2_in = dp.tile([128, NCL], dt.float8e4, name="ag2_in")


            def l_msg_chunk(Lh_src, nn):
                """L-message for lit chunk nn (512 lits) + AG1 kickoff."""
                sl = slice(512 * nn, 512 * (nn + 1))
                ps = psm.tile([128, 512], dt.float32, tag="m",
                              name=f"lm1_{nn}")
                nc.tensor.matmul(ps, C["lm_w1t"], Lh_src[:, sl],
                                 start=True, stop=True)
                nc.scalar.activation(h1l[:, sl], ps, AF.Relu, bias=C["lm_b1"])
                ps = psm.tile([128, 512], dt.float32, tag="m",
                              name=f"lm2_{nn}")
                nc.tensor.matmul(ps, C["lm_w2t"], h1l[:, sl],
                                 start=True, stop=True)
                nc.scalar.activation(h2l[:, sl], ps, AF.Relu, bias=C["lm_b2"])
                for j in range(4 * nn, 4 * nn + 4):
                    ps = psm.tile([128, 128], dt.float32, tag="m",
                                  name=f"lm3_{nn}_{j}")
                    nc.tensor.matmul(ps, h2l[:, 128 * j:128 * (j + 1)],
                                     C["lm_w3t"], start=True, stop=True)
                    nc.scalar.activation(lpre_img[:, 128 * j:128 * (j + 1)], ps,
                                         AF.Copy, scale=C["rowsc"][:, j:j + 1])
                if nn == 1:
                    nc.gpsimd.dma_start(out=ag1_in, in_=lpre_img)
                    nc.gpsimd.collective_compute(
                        "AllGather", ALU.bypass, replica_groups=rg,
                        ins=[ag1_in.opt()], outs=[ag1_out.ap().opt()])

            def land_ag1():
                for kk in range(NCORES):
                    nc.gpsimd.dma_start(
                        out=lpre_full[:, NL * kk:NL * (kk + 1)],
                        in_=ag1_out.ap()[kk])

            def c_msg_chunk(nn):
                """C-message for clause chunk nn (512 clauses)."""
                sl = slice(512 * nn, 512 * (nn + 1))
                ps = psm.tile([128, 512], dt.float32, tag="m",
                              name=f"cm1_{nn}")
                nc.tensor.matmul(ps, C["cm_w1t"], Ch[:, sl],
                                 start=True, stop=True)
                nc.scalar.activation(h1c[:, sl], ps, AF.Relu, bias=C["cm_b1"])
                ps = psm.tile([128, 512], dt.float32, tag="m",
                              name=f"cm2_{nn}")
                nc.tensor.matmul(ps, C["cm_w2t"], h1c[:, sl],
                                 start=True, stop=True)
                nc.scalar.activation(h2c[:, sl], ps, AF.Relu, bias=C["cm_b2"])
                for j in range(4 * nn, 4 * nn + 4):
                    ps = psm.tile([128, 128], dt.float32, tag="m",
                                  name=f"cm3_{nn}_{j}")
                    nc.tensor.matmul(ps, h2c[:, 128 * j:128 * (j + 1)],
                                     C["cm_w3t"], start=True, stop=True)
                    nc.scalar.activation(cpre_img[:, 128 * j:128 * (j + 1)], ps,
                                         AF.Copy, scale=C["colsc"][:, j:j + 1])

            def ag2_kickoff():
                nc.gpsimd.dma_start(out=ag2_in, in_=cpre_img)
                nc.gpsimd.collective_compute(
                    "AllGather", ALU.bypass, replica_groups=rg,
                    ins=[ag2_in.opt()], outs=[ag2_out.ap().opt()])

            def land_ag2():
                for kk in range(NCORES):
                    nc.gpsimd.dma_start(
                        out=cpre_full[:, NCL * kk:NCL * (kk + 1)],
                        in_=ag2_out.ap()[kk])

            def lstm_chunk(which, cc, Lh_src=None, Lh_dst=None):
                """LSTM gate + state update for one 512-col chunk."""
                sl = slice(512 * cc, 512 * (cc + 1))
                gts = []
                for g in range(4):
                    gs = slice(128 * g, 128 * (g + 1))
                    ps = psg.tile([128, 512], dt.float32, tag=f"g{g % 2}",
                                  name=f"ps_{which}_{cc}_{g}")
                    if which == "c":
                        nc.tensor.matmul(ps, C["cu_wt"][:, gs], lcs[:, sl],
                                         start=True, stop=False,
                                         skip_group_check=True)
                        nc.tensor.matmul(ps, C["cu_ut"][:, gs], Ch[:, sl],
                                         start=False, stop=False,
                                         skip_group_check=True)
                        nc.tensor.matmul(ps, C["u1"][0:1, gs],
                                         C["scol"][0:1, sl],
                                         start=False, stop=True,
                                         skip_group_check=True)
                        bias = C["cu_b"][:, g:g + 1]
                    else:
                        flip_sl = slice(512 * (1 - cc), 512 * (2 - cc))
                        nc.tensor.matmul(ps, C["lu_wcl"][:, gs], cls[:, sl],
                                         start=True, stop=False,
                                         skip_group_check=True)
                        nc.tensor.matmul(ps, C["lu_wfl"][:, gs],
                                         Lh_src[:, flip_sl],
                                         start=False, stop=False,
                                         skip_group_check=True)
                        nc.tensor.matmul(ps, C["lu_ut"][:, gs], Lh_src[:, sl],
                                         start=False, stop=False,
                                         skip_group_check=True)
                        nc.tensor.matmul(ps, C["u2"][0:1, gs],
                                         C["srow"][0:1, sl],
                                         start=False, stop=True,
                                         skip_group_check=True)
                        bias = C["lu_b"][:, g:g + 1]
                    gt = kp.tile([128, 512], dt.float32, tag=f"gate{g}",
                                 bufs=1, name=f"gt_{which}_{cc}_{g}")
                    nc.scalar.activation(gt, ps,
                                         AF.Tanh if g == 2 else AF.Sigmoid,
                                         bias=bias)
                    gts.append(gt)
                cell = Cc if which == "c" else Lc
                hout = Ch if which == "c" else Lh_dst
                t1 = kp.tile([128, 512], dt.float32, tag="t1", bufs=1,
                             name=f"t1_{which}_{cc}")
                t2 = kp.tile([128, 512], dt.float32, tag="t2", bufs=1,
                             name=f"t2_{which}_{cc}")
                nc.vector.tensor_tensor(out=t1, in0=gts[1], in1=cell[:, sl],
                                        op=ALU.mult)
                nc.vector.tensor_tensor(out=t2, in0=gts[0], in1=gts[2],
                                        op=ALU.mult)
                nc.vector.tensor_tensor(out=cell[:, sl], in0=t1, in1=t2,
                                        op=ALU.add)
                t3 = kp.tile([128, 512], dt.float32, tag="t3", bufs=1,
                             name=f"t3_{which}_{cc}")
                nc.scalar.activation(t3, cell[:, sl], AF.Tanh)
                nc.vector.tensor_tensor(out=hout[:, sl], in0=gts[3], in1=t3,
                                        op=ALU.mult)

            # ---- prologue: L message of round 0 ----
            for nn in range(2):
                l_msg_chunk(Lh_pp[0], nn)

            for r in range(rounds):
                Lh = Lh_pp[r % 2]
                Lh_new = Lh_pp[(r + 1) % 2]
                land_ag1()

                # ===== dir-1, phase-major (phase h = which AG1 half it needs),
                # with the C side inlined after each chunk completes =====
                ps1 = [psd.tile([128, 512], dt.float32, tag=f"d{nn}",
                                name=f"ps1_{r}_{nn}") for nn in range(4)]
                for h in range(2):
                    for nn in range(4):
                        for s2 in range(2):
                            b1t = bp.tile([128, 16 * 512], dt.float8e4,
                                          tag="b1", bufs=6,
                                          name=f"b1_{r}_{nn}_{h}_{s2}")
                            nc.sync.dma_start(
                                out=b1t,
                                in_=b1.ap()[nn, h][:, 16 * 512 * s2:
                                                   16 * 512 * (s2 + 1)])
                            for ttp in range(8):
                                tt = 16 * s2 + 2 * ttp
                                t = 8 * (tt // 4) + 4 * h + tt % 4
                                lhsT = lpre_full[:, 128 * t:128 * (t + 2)] \
                                    .rearrange("p (e d) -> p e d", e=2)
                                rhs = b1t[:, 1024 * ttp:1024 * (ttp + 1)] \
                                    .rearrange("p (e c) -> p e c", e=2)
                                nc.tensor.matmul(
                                    ps1[nn], lhsT, rhs,
                                    start=(h == 0 and tt == 0),
                                    stop=(h == 1 and tt == 30),
                                    perf_mode=mybir.MatmulPerfMode.DoubleRow,
                                    skip_group_check=True)
                        if h == 1:
                            sl = slice(512 * nn, 512 * (nn + 1))
                            nc.vector.tensor_tensor(out=lcs[:, sl],
                                                    in0=ps1[nn],
                                                    in1=C["colb"][:, sl],
                                                    op=ALU.mult)
                            lstm_chunk("c", nn)
                            c_msg_chunk(nn)
                            if nn == 3:
                                ag2_kickoff()

                land_ag2()
                # ===== dir-2 + L side, chunk-major =====
                for nn in range(2):
                    ps2 = psd.tile([128, 512], dt.float32, tag=f"d{nn}",
                                   name=f"ps2_{r}_{nn}")
                    for h in range(2):
                        for s in range(2):
                            for s2 in range(2):
                                b2t = bp.tile([128, 16 * 512], dt.float8e4,
                                              tag="b2", bufs=6,
                                              name=f"b2_{r}_{nn}_{h}_{s}_{s2}")
                                nc.sync.dma_start(
                                    out=b2t,
                                    in_=b2.ap()[nn, h, s][:, 16 * 512 * s2:
                                                          16 * 512 * (s2 + 1)])
                                for ttp in range(8):
                                    tt = 16 * s2 + 2 * ttp
                                    T = 16 * (4 * s + tt // 8) + 8 * h + tt % 8
                                    lhsT = cpre_full[:, 128 * T:128 * (T + 2)] \
                                        .rearrange("p (e d) -> p e d", e=2)
                                    rhs = b2t[:, 1024 * ttp:1024 * (ttp + 1)] \
                                        .rearrange("p (e c) -> p e c", e=2)
                                    nc.tensor.matmul(
                                        ps2, lhsT, rhs,
                                        start=(h == 0 and s == 0 and tt == 0),
                                        stop=(h == 1 and s == 1 and tt == 30),
                                        perf_mode=mybir.MatmulPerfMode.DoubleRow,
                                        skip_group_check=True)
                    sl = slice(512 * nn, 512 * (nn + 1))
                    nc.vector.tensor_tensor(out=cls[:, sl], in0=ps2,
                                            in1=C["rowb"][:, sl], op=ALU.mult)
                    lstm_chunk("l", nn, Lh_src=Lh, Lh_dst=Lh_new)
                    if r < rounds - 1:
                        l_msg_chunk(Lh_new, nn)

            # ===== vote MLP (bias of last layer added host-side) =====
            Lh_fin = Lh_pp[rounds % 2]
            vote_sb = wp.tile([1, NL], dt.float32, name="vote_sb")
            for nn in range(2):
                sl = slice(512 * nn, 512 * (nn + 1))
                ps = psm.tile([128, 512], dt.float32, tag="m", name=f"v1_{nn}")
                nc.tensor.matmul(ps, C["lv_w1t"], Lh_fin[:, sl],
                                 start=True, stop=True)
                nc.scalar.activation(h1l[:, sl], ps, AF.Relu, bias=C["lv_b1"])
                ps = psm.tile([128, 512], dt.float32, tag="m", name=f"v2_{nn}")
                nc.tensor.matmul(ps, C["lv_w2t"], h1l[:, sl],
                                 start=True, stop=True)
                nc.scalar.activation(h2l[:, sl], ps, AF.Relu, bias=C["lv_b2"])
                ps = psm.tile([1, 512], dt.float32, tag="m", name=f"v3_{nn}")
                nc.tensor.matmul(ps, C["lv_w3t"], h2l[:, sl],
                                 start=True, stop=True)
                nc.scalar.activation(vote_sb[0:1, sl], ps, AF.Copy)
            nc.scalar.dma_start(out=vote_out.ap(), in_=vote_sb)

    nc.compile()
    return nc


# ---------------------------------------------------------------------------
# host-side input preparation
# ---------------------------------------------------------------------------

def prep_inputs(inputs):
    g = {k: np.asarray(v) for k, v in inputs.items()}
    lit_idx = g["lit_idx"].astype(np.int64)
    clause_idx = g["clause_idx"].astype(np.int64)

    B = np.zeros((NL_TOT, NCL_TOT), np.bool_)
    B[lit_idx, clause_idx] = True
    degc = B.sum(0).astype(np.float32)
    degl = B.sum(1).astype(np.float32)
    col = (np.float32(1.0) / (np.sqrt(degc) + np.float32(1e-6))).astype(np.float32)
    row = (np.float32(1.0) / (np.sqrt(degl) + np.float32(1e-6))).astype(np.float32)
    # degree-0 rows/cols of A are structurally zero: clamp their scales so the
    # gained fp8 messages stay finite (mathematically identical result)
    col = np.where(degc > 0, col, np.float32(0)).astype(np.float32)
    row = np.where(degl > 0, row, np.float32(0)).astype(np.float32)

    # permuted lit order: core k <- [512k..512k+512) u [4096+512k..4096+512k+512)
    lit_order = np.concatenate(
        [np.concatenate([np.arange(512 * k, 512 * (k + 1)),
                         NV + np.arange(512 * k, 512 * (k + 1))])
         for k in range(NCORES)])
    Bu = B.astype(np.uint8) * FP8_ONE
    Bp = Bu[lit_order]                      # [8192, 16384] permuted rows
    row_p = row[lit_order]

    Bf32 = B.astype(np.float32)
    s_c = row @ Bf32                        # [NCL_TOT]
    scol_full = (col * s_c).astype(np.float32)
    s_l = Bf32 @ col
    srow_full = ((row * s_l).astype(np.float32))[lit_order]

    def b(x):
        return np.ascontiguousarray(np.asarray(x, np.float32)).astype(bf16)

    common = {
        "lm_w1t": b(g["lm_w1"].T), "lm_w2t": b(g["lm_w2"].T), "lm_w3t": b(g["lm_w3"].T),
        "cm_w1t": b(g["cm_w1"].T), "cm_w2t": b(g["cm_w2"].T), "cm_w3t": b(g["cm_w3"].T),
        "lv_w1t": b(g["lv_w1"].T), "lv_w2t": b(g["lv_w2"].T), "lv_w3t": b(g["lv_w3"].T),
        "lm_b1": np.asarray(g["lm_b1"], np.float32).reshape(128, 1),
        "lm_b2": np.asarray(g["lm_b2"], np.float32).reshape(128, 1),
        "cm_b1": np.asarray(g["cm_b1"], np.float32).reshape(128, 1),
        "cm_b2": np.asarray(g["cm_b2"], np.float32).reshape(128, 1),
        "lv_b1": np.asarray(g["lv_b1"], np.float32).reshape(128, 1),
        "lv_b2": np.asarray(g["lv_b2"], np.float32).reshape(128, 1),
        "cu_wt": b(g["cu_wih"].T), "cu_ut": b(g["cu_whh"].T),
        "lu_wcl": b(g["lu_wih"][:, :D].T), "lu_wfl": b(g["lu_wih"][:, D:].T),
        "lu_ut": b(g["lu_whh"].T),
        "cu_b": np.asarray(g["cu_bih"] + g["cu_bhh"], np.float32).reshape(4, 128).T.copy(),
        "lu_b": np.asarray(g["lu_bih"] + g["lu_bhh"], np.float32).reshape(4, 128).T.copy(),
        "u1": b(np.asarray(g["cu_wih"], np.float32) @ np.asarray(g["lm_b3"], np.float32)).reshape(1, 512),
        "u2": b(np.asarray(g["lu_wih"], np.float32)[:, :D] @ np.asarray(g["cm_b3"], np.float32)).reshape(1, 512),
        "lh0": np.ascontiguousarray(np.broadcast_to(
            np.asarray(g["L_init_w"][:, 0] + g["L_init_b"], np.float32)[:, None],
            (128, NL))).astype(bf16),
        "ch0": np.ascontiguousarray(np.broadcast_to(
            np.asarray(g["C_init_w"][:, 0] + g["C_init_b"], np.float32)[:, None],
            (128, NCL))).astype(bf16),
    }

    in_maps = []
    for k in range(NCORES):
        lsl = slice(NL * k, NL * (k + 1))
        csl = slice(NCL * k, NCL * (k + 1))
        # b1: Bp[:, csl] is [t(64)*128p rows, nn(4)*512c cols]
        #     [kk(8), jh(2), jj(4), p, nn, c] -> [nn, jh, p, kk, jj, c]
        X = Bp[:, csl].reshape(8, 2, 4, 128, 4, 512)
        b1k = np.ascontiguousarray(X.transpose(4, 1, 3, 0, 2, 5)).reshape(
            4, 2, 128, 32 * 512).view(f8)
        # b2: Bp[lsl, :].T is [T(128)*128p rows, nn(2)*512l cols]
        #     [ks(2), kk(4), h(2), j2(8), p, nn, l] -> [nn, h, ks, p, kk, j2, l]
        Y = Bp[lsl, :].T.reshape(2, 4, 2, 8, 128, 2, 512)
        b2k = np.ascontiguousarray(Y.transpose(5, 2, 0, 4, 1, 3, 6)).reshape(
            2, 2, 2, 128, 32 * 512).view(f8)
        m = dict(common)
        m.update({
            "b1": b1k,
            "b2": b2k,
            "scol": scol_full[csl].astype(bf16).reshape(1, NCL),
            "srow": srow_full[lsl].astype(bf16).reshape(1, NL),
            "colb": np.ascontiguousarray(
                np.broadcast_to(col[csl][None, :] / GAIN, (128, NCL))).astype(bf16),
            "rowb": np.ascontiguousarray(
                np.broadcast_to(row_p[lsl][None, :] / GAIN, (128, NL))).astype(bf16),
            "rowsc": np.ascontiguousarray(
                GAIN * row_p[lsl].reshape(8, 128).T).astype(np.float32),
            "colsc": np.ascontiguousarray(
                GAIN * col[csl].reshape(16, 128).T).astype(np.float32),
        })
        in_maps.append(m)
    return in_maps


def selfcheck_layouts(in_maps, lit_idx, clause_idx):
    """Random probes: device-layout b1/b2 entries vs the raw B matrix."""
    B = np.zeros((NL_TOT, NCL_TOT), np.uint8)
    B[lit_idx, clause_idx] = FP8_ONE
    lit_order = np.concatenate(
        [np.concatenate([np.arange(512 * k, 512 * (k + 1)),
                         NV + np.arange(512 * k, 512 * (k + 1))])
         for k in range(NCORES)])
    Bp = B[lit_order]
    rng = np.random.default_rng(1)
    for k in (0, 3):
        b1k = in_maps[k]["b1"].view(np.uint8).reshape(4, 2, 128, 32, 512)
        for _ in range(50):
            nn, h, p, tt, c = (rng.integers(4), rng.integers(2), rng.integers(128),
                               rng.integers(32), rng.integers(512))
            t = 8 * (tt // 4) + 4 * h + tt % 4
            want = Bp[128 * t + p, 2048 * k + 512 * nn + c]
            assert b1k[nn, h, p, tt, c] == want, (k, nn, h, p, tt, c)
        b2k = in_maps[k]["b2"].view(np.uint8).reshape(2, 2, 2, 128, 32, 512)
        for _ in range(50):
            nn, h, s, p, tt, c = (rng.integers(2), rng.integers(2), rng.integers(2),
                                  rng.integers(128), rng.integers(32), rng.integers(512))
            T = 16 * (4 * s + tt // 8) + 8 * h + tt % 8
            want = Bp[1024 * k + 512 * nn + c, 128 * T + p]
            assert b2k[nn, h, s, p, tt, c] == want, (k, nn, h, s, p, tt, c)


_PROGRAM_CACHE = {}


def _get_program(rounds):
    if rounds not in _PROGRAM_CACHE:
        _PROGRAM_CACHE[rounds] = build_program(rounds)
    return _PROGRAM_CACHE[rounds]


def run_device(inputs, trace=False, rounds=None, **kw):
    if rounds is None:
        rounds = int(inputs.get("n_rounds", 16))
    in_maps = prep_inputs(inputs)
    nc = _get_program(rounds)
    res = bass_utils.run_bass_kernel_spmd(
        nc, in_maps, core_ids=list(range(NCORES)), trace=trace, **kw)
    return res


def assemble_votes(res_results, lv_b3):
    votes = np.stack([np.asarray(res_results[k]["vote"]).reshape(NL)
                      for k in range(NCORES)])   # [8, 1024]
    vote = votes + np.float32(lv_b3)
    pos = vote[:, :512].reshape(NV)              # var v -> core v//512
    neg = vote[:, 512:].reshape(NV)
    vj = np.stack([pos, neg], axis=1)            # [4096, 2]
    return vj.reshape(32, -1).mean(axis=1).astype(np.float32)


def kernel(**inputs) -> np.ndarray:
    res = run_device(inputs)
    return assemble_votes(res.results, np.asarray(inputs["lv_b3"]).reshape(-1)[0])
